# revision 16
# baseline (speedup 1.0000x reference)
"""CNNSummarizer (CNN encoder + 2-layer LSTM decoder + vocab projection) on 8 trn2 cores.

Sharding:
  - encoder: data-parallel over batch (4 batches per core); one AllGather of the
    per-batch encoder contribution to the LSTM-1 input preactivation (32KB).
  - LSTM recurrence: replicated on all 8 cores (small-collective latency makes
    per-step sharding a loss).
  - vocab projection (H -> V GEMM): column-sharded, 4000 vocab per core.

The LSTM input preactivations X@w_ih1 are precomputed for all steps (teacher
forcing) as one big GEMM into a DRAM buffer packed as (t, 32*gateblock+batch,
512); the recurrent h@w_hh GEMMs then run as 4-way column-tiled matmuls
(stationary = h^T K-chunk, streaming = weight rows) producing gates directly in
a (gateblock*32+b, d) PSUM layout, gate order host-permuted to [i, f, o, g] so
sigmoid runs as one 96-partition op.

Host-side work is limited to input marshalling: dtype casts of index tensors,
weight transposes/permutations, and the final gather/reshape of the output.
"""

import math
from contextlib import ExitStack

import numpy as np

import concourse.bacc as bacc
import concourse.bass as bass
import concourse.mybir as mybir
import concourse.tile as tile
from concourse.masks import make_identity

V, E, H, F = 32000, 256, 512, 256
FS = (3, 4, 5)
B, S, T = 32, 512, 64
NCORES = 8
BL = B // NCORES          # batches per core
VS = V // NCORES          # vocab shard per core
TT = T - 1                # decode steps actually computed
G4 = 4 * H                # 2048 gates

dt = mybir.dt
F32 = dt.float32
F32R = dt.float32r
AF = mybir.ActivationFunctionType
ALU = mybir.AluOpType
AX = mybir.AxisListType


def _r(ap):
    """View an fp32 AP as float32r for full-rate PE matmuls."""
    return ap.bitcast(F32R)


def build(tt=TT, vs=VS, trace_sim=False):
    """Build the per-core program. All 8 cores run the same NEFF; sharding comes
    from per-core input values."""
    R = tt * B                       # rows of the (t, b) decode matrix
    NM = math.ceil(R / 128)          # m-chunks of decode rows
    NCH = NM                         # dec-token gather chunks (128 tokens each)
    RPAD = NM * 128
    NV = math.ceil(vs / 512)

    nc = bacc.Bacc("TRN2", target_bir_lowering=False, debug=False,
                   num_devices=NCORES)

    def inp(name, shape, dtype=F32):
        return nc.dram_tensor(name, list(shape), dtype, kind="ExternalInput").ap()

    src_idx = inp("src_idx", (128, (BL * S) // 128), dt.int32)
    dec_idx = inp("dec_idx", (128, NCH), dt.int32)
    enc_emb = inp("enc_emb", (V, E))
    dec_emb = inp("dec_emb", (V, E))
    wconv = {k: inp(f"wconv{k}", (128, k * 4 * 128), F32R) for k in FS}
    bconv = inp("bconv", (128, 2 * len(FS)))   # col = fc*3 + k_idx
    fc1T = inp("fc1T", (128, 6 * H), F32R)
    fc1b = inp("fc1b", (1, H), F32R)
    fc2T = inp("fc2T", (128, 4 * H), F32R)
    fc2b = inp("fc2b", (1, H), F32R)
    WdT = inp("WdT", (128, 2 * G4), F32R)
    WeT = inp("WeT", (128, 4 * G4), F32R)
    b1row = inp("b1row", (1, G4), F32R)
    b2pack = inp("b2pack", (128, H), F32R)
    whh1T = inp("whh1T", (128, 4 * G4), dt.bfloat16)
    wih2T = inp("wih2T", (128, 4 * G4), dt.bfloat16)
    whh2T = inp("whh2T", (128, 4 * G4), dt.bfloat16)
    owT = inp("owT", (H, vs), F32R)
    obrow = inp("obrow", (1, vs), F32R)

    out_dram = nc.dram_tensor("logits_sh", [R, vs], F32,
                              kind="ExternalOutput").ap()

    with tile.TileContext(nc, trace_sim=trace_sim) as tc:
        with ExitStack() as ctx:
            dram = ctx.enter_context(tc.tile_pool(name="dram", bufs=1,
                                                  space="DRAM"))
            xih_dram = dram.tile([tt, 128, H], F32R)
            cc_in = dram.tile([BL, G4], F32R)
            cc_out = dram.tile([B, G4], F32R, addr_space="Shared")

            const = ctx.enter_context(tc.tile_pool(name="const", bufs=1))
            identF = const.tile([128, 128], F32)
            make_identity(nc, identF[:])
            ident = const.tile([128, 128], F32R)
            nc.vector.tensor_copy(ident[:], identF[:])
            scrF = const.tile([128, 128], F32)
            nc.vector.memset(scrF[:], 0.0)
            zpad = const.tile([128, 8], F32R)
            nc.vector.tensor_copy(zpad[:], scrF[:, 0:8])
            nc.vector.memset(scrF[0:1, :], 1.0)
            ones = const.tile([1, 128], F32R)
            nc.vector.tensor_copy(ones[:], scrF[0:1, :])
            stack4I = const.tile([32, 128], F32R)
            for tau in range(4):
                nc.vector.tensor_copy(stack4I[:, 32 * tau:32 * tau + 32],
                                      ident[0:32, 0:32])
            b2_sb = const.tile([128, H], F32R)
            nc.sync.dma_start(b2_sb[:], b2pack)

            # h2^T lives across recurrence + vocab phases
            big = ctx.enter_context(tc.tile_pool(name="big", bufs=1))
            h2T_all = big.tile([128, 4 * RPAD], F32R)   # [kc] blocks of h2^T

            # =========================================================
            # Phase 1: encoder (my BL batches) + Xdec GEMM (all rows)
            # =========================================================
            with ExitStack() as p1:
                wpool1 = p1.enter_context(tc.tile_pool(name="wpool1", bufs=1))
                gpool = p1.enter_context(tc.tile_pool(name="gpool", bufs=3))
                tpp = p1.enter_context(tc.tile_pool(name="tpp", bufs=2,
                                                    space="PSUM"))
                cps = p1.enter_context(tc.tile_pool(name="cps", bufs=3,
                                                    space="PSUM"))
                fps = p1.enter_context(tc.tile_pool(name="fps", bufs=2,
                                                    space="PSUM"))
                p1e = ExitStack()
                encp = p1e.enter_context(tc.tile_pool(name="encp", bufs=1))
                wconv_sb = {}
                for k in FS:
                    wk = encp.tile([128, k * 4 * 128], F32R,
                                   name=f"wconv{k}_sb")
                    nc.sync.dma_start(
                        wk[:], wconv[k])
                    wconv_sb[k] = wk
                bconv_sb = encp.tile([128, 2 * len(FS)], F32)
                nc.sync.dma_start(bconv_sb[:], bconv)
                fc1T_sb = encp.tile([128, 6 * H], F32R)
                nc.sync.dma_start(fc1T_sb[:], fc1T)
                fc2T_sb = encp.tile([128, 4 * H], F32R)
                nc.sync.dma_start(fc2T_sb[:], fc2T)
                fc1b_sb = encp.tile([1, H], F32R)
                nc.sync.dma_start(fc1b_sb[:], fc1b)
                fc2b_sb = encp.tile([1, H], F32R)
                nc.sync.dma_start(fc2b_sb[:], fc2b)
                WdT_sb = wpool1.tile([128, 2 * G4], F32R)
                nc.sync.dma_start(WdT_sb[:], WdT)
                WeT_sb = encp.tile([128, 4 * G4], F32R)
                nc.sync.dma_start(WeT_sb[:], WeT)
                b1_sb = encp.tile([1, G4], F32R)
                nc.sync.dma_start(b1_sb[:], b1row)
                idx_s_sb = encp.tile([128, (BL * S) // 128], dt.int32)
                nc.sync.dma_start(idx_s_sb[:], src_idx)
                idx_d_sb = wpool1.tile([128, NCH], dt.int32)
                nc.sync.dma_start(idx_d_sb[:], dec_idx)

                XPAD = BL * S + 8
                xT_sb = encp.tile([128, 2 * XPAD], F32R)        # [ec] blocks
                dembT_sb = wpool1.tile([128, 2 * RPAD], F32R)   # [ec] blocks

                def evict(dst, src, parity):
                    if parity % 2 == 0:
                        nc.vector.tensor_copy(dst, src)
                    else:
                        nc.scalar.copy(dst, src)

                # ---- gather + transpose embeddings ----
                def gather_transpose(idx_sb, nch, table, dstT, dpad):
                    for ch in range(nch):
                        gt = gpool.tile([128, E], F32, tag="gath")
                        nc.gpsimd.indirect_dma_start(
                            out=gt[:], out_offset=None, in_=table,
                            in_offset=bass.IndirectOffsetOnAxis(
                                ap=idx_sb[:, ch:ch + 1], axis=0))
                        for ec in range(2):
                            tp = tpp.tile([128, 128], F32, tag="tp",
                                          space="PSUM")
                            nc.tensor.transpose(
                                tp[:], gt[:, 128 * ec:128 * ec + 128],
                                ident[:].bitcast(F32))
                            evict(dstT[:, ec * dpad + 128 * ch:
                                       ec * dpad + 128 * ch + 128],
                                  tp[:], ch + ec)

                gather_transpose(idx_s_sb, (BL * S) // 128, enc_emb, xT_sb, XPAD)
                for ec in range(2):
                    nc.vector.tensor_copy(
                        xT_sb[:, ec * XPAD + BL * S: (ec + 1) * XPAD], zpad[:])
                gather_transpose(idx_d_sb, NCH, dec_emb, dembT_sb, RPAD)

                # ---- conv + maxpool + relu(+bias) -> pooled ----
                pooled = encp.tile([128, 6 * BL], F32R)
                for ki, k in enumerate(FS):
                    for fc in range(2):
                        for b in range(BL):
                            ps = cps.tile([128, 512], F32, tag="conv",
                                          space="PSUM")
                            first = True
                            for j in range(k):
                                for ec in range(2):
                                    lhs = wconv_sb[k][
                                        :, (j * 4 + ec * 2 + fc) * 128:
                                        (j * 4 + ec * 2 + fc) * 128 + 128]
                                    rhs = xT_sb[:, ec * XPAD + 512 * b + j:
                                                ec * XPAD + 512 * b + j + 512]
                                    nc.tensor.matmul(
                                        ps[:], _r(lhs), _r(rhs), start=first,
                                        stop=(j == k - 1 and ec == 1))
                                    first = False
                            kc = ki * 2 + fc
                            nc.vector.tensor_reduce(
                                pooled[:, BL * kc + b: BL * kc + b + 1],
                                ps[:, 0:S - k + 1], axis=AX.X, op=ALU.max)
                for ki in range(len(FS)):
                    for fc in range(2):
                        kc = ki * 2 + fc
                        nc.scalar.activation(
                            pooled[:, BL * kc: BL * kc + BL],
                            pooled[:, BL * kc: BL * kc + BL],
                            AF.Relu, bias=bconv_sb[:, fc * 3 + ki: fc * 3 + ki + 1])

                # ---- fc1 -> relu -> fc2 -> Xenc -> AllGather ----
                ps1 = fps.tile([BL, H], F32, tag="f", space="PSUM")
                for kc in range(6):
                    nc.tensor.matmul(ps1[:], _r(pooled[:, BL * kc: BL * kc + BL]),
                                     _r(fc1T_sb[:, H * kc: H * kc + H]),
                                     start=(kc == 0), stop=False)
                nc.tensor.matmul(ps1[:], _r(ones[0:1, 0:BL]), _r(fc1b_sb[:]),
                                 start=False, stop=True)
                h1e = encp.tile([BL, H], F32)
                nc.scalar.activation(h1e[:], ps1[:], AF.Relu)

                h1eT = encp.tile([128, 4 * BL], F32R)
                for kc in range(4):
                    tp = tpp.tile([128, 128], F32, tag="tp", space="PSUM")
                    nc.tensor.transpose(tp[0:128, 0:BL],
                                        h1e[:, 128 * kc:128 * kc + 128],
                                        ident[0:BL, 0:BL].bitcast(F32))
                    nc.vector.tensor_copy(h1eT[:, BL * kc:BL * kc + BL],
                                          tp[0:128, 0:BL])

                ps2 = fps.tile([BL, H], F32, tag="f", space="PSUM")
                for kc in range(4):
                    nc.tensor.matmul(ps2[:], _r(h1eT[:, BL * kc:BL * kc + BL]),
                                     _r(fc2T_sb[:, H * kc:H * kc + H]),
                                     start=(kc == 0), stop=False)
                nc.tensor.matmul(ps2[:], _r(ones[0:1, 0:BL]), _r(fc2b_sb[:]),
                                 start=False, stop=True)
                enc_sb = encp.tile([BL, H], F32)
                nc.vector.tensor_copy(enc_sb[:], ps2[:])

                encT = encp.tile([128, 4 * BL], F32R)
                for kc in range(4):
                    tp = tpp.tile([128, 128], F32, tag="tp", space="PSUM")
                    nc.tensor.transpose(tp[0:128, 0:BL],
                                        enc_sb[:, 128 * kc:128 * kc + 128],
                                        ident[0:BL, 0:BL].bitcast(F32))
                    nc.vector.tensor_copy(encT[:, BL * kc:BL * kc + BL],
                                          tp[0:128, 0:BL])

                xe_sb = encp.tile([BL, G4], F32R)
                for n in range(4):
                    ps = fps.tile([BL, 512], F32, tag="f", space="PSUM")
                    for kc in range(4):
                        nc.tensor.matmul(
                            ps[:], _r(encT[:, BL * kc:BL * kc + BL]),
                            _r(WeT_sb[:, kc * G4 + 512 * n:
                                      kc * G4 + 512 * n + 512]),
                            start=(kc == 0), stop=False)
                    nc.tensor.matmul(ps[:], _r(ones[0:1, 0:BL]),
                                     _r(b1_sb[:, 512 * n:512 * n + 512]),
                                     start=False, stop=True)
                    nc.vector.tensor_copy(xe_sb[:, 512 * n:512 * n + 512], ps[:])
                nc.sync.dma_start(cc_in[:], xe_sb[:])
                p1e.close()

                nc.gpsimd.collective_compute(
                    "AllGather", ALU.bypass,
                    replica_groups=[list(range(NCORES))],
                    ins=[cc_in.opt()], outs=[cc_out.opt()])

                xeall_sb = wpool1.tile([B, G4], F32R)
                nc.sync.dma_start(xeall_sb[:], cc_out[:])

                # ---- Xdec GEMM + Xenc broadcast-add -> packed xih_dram ----
                xdpool = p1.enter_context(tc.tile_pool(name="xdpool", bufs=2))
                for m in range(NM):
                    tm = min(4, tt - 4 * m)
                    Mm = 32 * tm
                    xd_sb = xdpool.tile([128, G4], F32R, tag="xd_sb")
                    for n in range(4):
                        ps = fps.tile([128, 512], F32, tag="f", space="PSUM")
                        for ec in range(2):
                            nc.tensor.matmul(
                                ps[0:Mm, :],
                                _r(dembT_sb[:, ec * RPAD + 128 * m:
                                            ec * RPAD + 128 * m + Mm]),
                                _r(WdT_sb[:, ec * G4 + 512 * n:
                                          ec * G4 + 512 * n + 512]),
                                start=(ec == 0), stop=False)
                        nc.tensor.matmul(ps[0:Mm, :], _r(stack4I[0:32, 0:Mm]),
                                         _r(xeall_sb[:, 512 * n:512 * n + 512]),
                                         start=False, stop=True)
                        evict(xd_sb[0:Mm, 512 * n:512 * n + 512],
                              ps[0:Mm, :], n)
                    for tau in range(tm):
                        dst = xih_dram[4 * m + tau].rearrange(
                            "(j b) d -> b j d", j=4)
                        nc.sync.dma_start(dst,
                                          xd_sb[32 * tau:32 * tau + 32, :])

            # =========================================================
            # Phase 2: recurrence with packed gate PSUM, col-tiled GEMMs
            # gate-block order [i, f, o, g] on psum partitions [0:32,...]
            # =========================================================
            rw = ctx.enter_context(tc.tile_pool(name="rw", bufs=1))
            whh1_sb = rw.tile([128, 4 * G4], dt.bfloat16)
            nc.sync.dma_start(whh1_sb[:], whh1T)
            wih2_sb = rw.tile([128, 4 * G4], dt.bfloat16)
            nc.sync.dma_start(wih2_sb[:], wih2T)
            whh2_sb = rw.tile([128, 4 * G4], dt.bfloat16)
            nc.sync.dma_start(whh2_sb[:], whh2T)

            with ExitStack() as p2:
                rp = p2.enter_context(tc.tile_pool(name="rp", bufs=2))
                xp = p2.enter_context(tc.tile_pool(name="xp", bufs=3))
                rps = p2.enter_context(tc.tile_pool(name="rps", bufs=2,
                                                    space="PSUM"))
                tps = p2.enter_context(tc.tile_pool(name="tps", bufs=2,
                                                    space="PSUM"))

                c1 = rp.tile([32, H], F32, tag="c1")
                nc.vector.memset(c1[:], 0.0)
                c2 = rp.tile([32, H], F32, tag="c2")
                nc.vector.memset(c2[:], 0.0)

                def cell(ps_g, c_prev, tag):
                    """LSTM cell from packed-gate psum (128, H) -> (h, c_new).

                    The sigmoid/tanh ACT ops double as partition moves: each
                    reads its gate block [32j:32j+32] and writes a base-0 tile
                    (cross-partition activation is legal; 2-input DVE ops are
                    not, so all vector ops below stay partition-aligned)."""
                    si = rp.tile([32, H], F32, tag=f"si{tag}")
                    nc.scalar.activation(si[:], ps_g[0:32, :], AF.Sigmoid)
                    sf = rp.tile([32, H], F32, tag=f"sf{tag}")
                    nc.scalar.activation(sf[:], ps_g[32:64, :], AF.Sigmoid)
                    so = rp.tile([32, H], F32, tag=f"so{tag}")
                    nc.scalar.activation(so[:], ps_g[64:96, :], AF.Sigmoid)
                    tg = rp.tile([32, H], F32, tag=f"tg{tag}")
                    nc.scalar.activation(tg[:], ps_g[96:128, :], AF.Tanh)
                    nc.vector.tensor_mul(tg[:], si[:], tg[:])
                    nc.vector.tensor_mul(sf[:], sf[:], c_prev[:])
                    c_new = rp.tile([32, H], F32, tag=f"c{tag}")
                    nc.vector.tensor_add(c_new[:], tg[:], sf[:])
                    nc.scalar.activation(tg[:], c_new[:], AF.Tanh)
                    h = rp.tile([32, H], F32, tag=f"h{tag}")
                    nc.vector.tensor_mul(h[:], so[:], tg[:])
                    return h, c_new

                def transpose_state(h, dsts):
                    for kc in range(4):
                        tp = tps.tile([128, 32], F32, tag="tps", space="PSUM")
                        nc.tensor.transpose(tp[:], h[:, 128 * kc:128 * kc + 128],
                                            ident[0:32, 0:32].bitcast(F32))
                        for i, (dst, dst_base) in enumerate(dsts):
                            d = dst[:, dst_base(kc): dst_base(kc) + 32]
                            if (kc + i) % 2 == 0:
                                nc.vector.tensor_copy(d, tp[:])
                            else:
                                nc.scalar.copy(d, tp[:])

                def gemm_block(ps, stat, stat_base, w_sb, final):
                    for kc in range(4):
                        for j in range(4):
                            nc.tensor.matmul(
                                ps[32 * j:32 * j + 32, :],
                                stat[:, stat_base(kc): stat_base(kc) + 32],
                                w_sb[:, kc * G4 + 512 * j:
                                     kc * G4 + 512 * j + 512],
                                start=False,
                                stop=(final and kc == 3 and j == 3),
                                skip_group_check=True,
                                tile_position=(0, 32 * j))

                h1T = None
                h2Tb_prev = None
                for t in range(tt):
                    xih_t = xp.tile([128, H], F32R, tag="xih")
                    nc.sync.dma_start(xih_t[:], xih_dram[t])

                    ps_g1 = rps.tile([128, H], F32, tag="g1", space="PSUM")
                    nc.tensor.matmul(ps_g1[:], _r(ident[:]), _r(xih_t[:]),
                                     start=True, stop=(t == 0),
                                     skip_group_check=True)
                    if t > 0:
                        gemm_block(ps_g1, h1T, lambda kc: 32 * kc, whh1_sb, True)
                    h1, c1 = cell(ps_g1, c1, "1")

                    h1T = rp.tile([128, 128], dt.bfloat16, tag="h1T")
                    transpose_state(h1, [(h1T, lambda kc: 32 * kc)])

                    ps_g2 = rps.tile([128, H], F32, tag="g2", space="PSUM")
                    nc.tensor.matmul(ps_g2[:], _r(ident[:]), _r(b2_sb[:]),
                                     start=True, stop=False,
                                     skip_group_check=True)
                    gemm_block(ps_g2, h1T, lambda kc: 32 * kc, wih2_sb, t == 0)
                    if t > 0:
                        gemm_block(ps_g2, h2Tb_prev, lambda kc: 32 * kc,
                                   whh2_sb, True)
                    h2, c2 = cell(ps_g2, c2, "2")

                    h2Tb_prev = rp.tile([128, 128], dt.bfloat16, tag="h2Tb")
                    transpose_state(
                        h2, [(h2T_all, lambda kc, _t=t: kc * RPAD + 32 * _t),
                             (h2Tb_prev, lambda kc: 32 * kc)])

            # =========================================================
            # Phase 3: vocab projection, m-chunks of 128 (t,b) rows
            # =========================================================
            vw = ctx.enter_context(tc.tile_pool(name="vw", bufs=2))
            vo = ctx.enter_context(tc.tile_pool(name="vo", bufs=3))
            vps = ctx.enter_context(tc.tile_pool(name="vps", bufs=2,
                                                 space="PSUM"))
            obrow_sb = vw.tile([1, vs], F32R, tag="ob_row", bufs=1)
            nc.sync.dma_start(obrow_sb[:], obrow)
            for n in range(NV):
                nw = min(512, vs - 512 * n)
                wv = vw.tile([128, 4 * 512], F32R, tag="wv")
                for kc in range(4):
                    nc.sync.dma_start(wv[:, 512 * kc:512 * kc + nw],
                                      owT[128 * kc:128 * kc + 128,
                                          512 * n:512 * n + nw])
                for m in range(NM):
                    Mm = min(128, R - 128 * m)
                    ps = vps.tile([128, 512], F32, tag="vps", space="PSUM")
                    for kc in range(4):
                        nc.tensor.matmul(
                            ps[0:Mm, 0:nw],
                            _r(h2T_all[:, kc * RPAD + 128 * m:
                                       kc * RPAD + 128 * m + Mm]),
                            _r(wv[:, 512 * kc:512 * kc + nw]),
                            start=(kc == 0), stop=False)
                    nc.tensor.matmul(ps[0:Mm, 0:nw], _r(ones[0:1, 0:Mm]),
                                     _r(obrow_sb[:, 512 * n:512 * n + nw]),
                                     start=False, stop=True)
                    ob = vo.tile([128, 512], F32, tag="ob")
                    if m % 2 == 0:
                        nc.scalar.copy(ob[0:Mm, 0:nw], ps[0:Mm, 0:nw])
                    else:
                        nc.vector.tensor_copy(ob[0:Mm, 0:nw], ps[0:Mm, 0:nw])
                    nc.sync.dma_start(
                        out_dram[128 * m:128 * m + Mm, 512 * n:512 * n + nw],
                        ob[0:Mm, 0:nw])

    nc.compile()
    return nc


# =====================================================================
# Host side
# =====================================================================

def _bf16(a):
    import ml_dtypes
    return np.ascontiguousarray(a.astype(ml_dtypes.bfloat16))


def _chunk(a):
    """(c*128, X) -> (128, c*X): partition-chunked layout for SBUF tiles."""
    c = a.shape[0] // 128
    return np.ascontiguousarray(
        a.reshape(c, 128, -1).transpose(1, 0, 2).reshape(128, -1))


def host_prep(inputs, tt=TT, vs=VS):
    """Build per-core input maps from the full problem inputs."""
    R = tt * B
    NM = math.ceil(R / 128)
    f32 = lambda a: np.ascontiguousarray(np.asarray(a), dtype=np.float32)
    # gate permutation [i, f, o, g]
    perm = np.concatenate([np.arange(0, H), np.arange(H, 2 * H),
                           np.arange(3 * H, 4 * H), np.arange(2 * H, 3 * H)])

    src = np.asarray(inputs["src"])
    trg = np.asarray(inputs["trg"])

    w_ih1 = f32(inputs["w_ih1"])[perm]
    b1 = (f32(inputs["b_ih1"]) + f32(inputs["b_hh1"]))[perm][None, :]
    b2 = (f32(inputs["b_ih2"]) + f32(inputs["b_hh2"]))[perm]
    b2pack = np.ascontiguousarray(
        np.broadcast_to(b2.reshape(4, 1, H), (4, 32, H)).reshape(128, H))

    shared = {
        "enc_emb": f32(inputs["enc_emb"]),
        "dec_emb": f32(inputs["dec_emb"]),
        "bconv": np.ascontiguousarray(
            np.stack([f32(inputs[f"conv_b{k}"]).reshape(2, 128)[fc]
                      for fc in range(2) for k in FS], axis=1)),
        "fc1T": _chunk(f32(inputs["fc1_w"]).T),
        "fc1b": f32(inputs["fc1_b"])[None, :],
        "fc2T": _chunk(f32(inputs["fc2_w"]).T),
        "fc2b": f32(inputs["fc2_b"])[None, :],
        "WdT": _chunk(np.ascontiguousarray(w_ih1[:, :E].T)),
        "WeT": _chunk(np.ascontiguousarray(w_ih1[:, E:].T)),
        "b1row": b1, "b2pack": b2pack,
        "whh1T": _bf16(_chunk(np.ascontiguousarray(f32(inputs["w_hh1"])[perm].T))),
        "wih2T": _bf16(_chunk(np.ascontiguousarray(f32(inputs["w_ih2"])[perm].T))),
        "whh2T": _bf16(_chunk(np.ascontiguousarray(f32(inputs["w_hh2"])[perm].T))),
    }
    for k in FS:
        A = f32(inputs[f"conv_w{k}"]).transpose(2, 1, 0)   # (k, E, F)
        A = A.reshape(k, 2, 128, 2, 128).transpose(0, 1, 3, 2, 4)
        shared[f"wconv{k}"] = _chunk(A.reshape(k * 4 * 128, 128))

    dtoks = trg[:, :tt].T.reshape(-1).astype(np.int32)
    dtoks = np.concatenate([dtoks, np.zeros(NM * 128 - R, np.int32)])
    dec_idx = np.ascontiguousarray(dtoks.reshape(NM, 128).T)

    owT_full = np.ascontiguousarray(f32(inputs["out_w"]).T)   # (H, V)
    ob_full = f32(inputs["out_b"])

    in_maps = []
    for c in range(NCORES):
        stoks = src[BL * c: BL * (c + 1)].reshape(-1).astype(np.int32)
        m = dict(shared)
        m["src_idx"] = np.ascontiguousarray(stoks.reshape(-1, 128).T)
        m["dec_idx"] = dec_idx
        m["owT"] = np.ascontiguousarray(owT_full[:, vs * c: vs * (c + 1)])
        m["obrow"] = np.ascontiguousarray(ob_full[None, vs * c: vs * (c + 1)])
        in_maps.append(m)
    return in_maps


def assemble(results, tt=TT, vs=VS):
    """Gather per-core logit shards -> full (B, T, V) output."""
    out = np.zeros((B, T, V), dtype=np.float32)
    for c, res in enumerate(results):
        sh = np.asarray(res["logits_sh"]).reshape(tt, B, vs)
        out[:, 1:1 + tt, vs * c: vs * (c + 1)] = sh.transpose(1, 0, 2)
    return out


_CACHE = {}


def kernel(**inputs):
    if "nc" not in _CACHE:
        _CACHE["nc"] = build()
    nc = _CACHE["nc"]
    from concourse.bass_utils import run_bass_kernel_spmd
    in_maps = host_prep(inputs)
    res = run_bass_kernel_spmd(nc, in_maps, core_ids=list(range(NCORES)))
    return assemble(res.results)
